# revision 45
# baseline (speedup 1.0000x reference)
"""Causal multi-head attention (b=4, t=2048, d=1024, 16 heads) on 8 trn2 cores.

Sharding: data-parallel over batch (4) x tensor-parallel over head halves (2).
Each core handles one batch b and 8 heads. Projections run in fp32r (tf32-like)
matmuls; attention internals (scores, exp probs, PV, out-proj) run in bf16,
which removes the fp32r narrow-free-dim penalty and halves SBUF traffic.

Emission is software-pipelined at instruction granularity: the attention inner
loop (scores -> exp -> PV, which is Activation-engine paced) pops "filler"
units -- projection matmuls for the next t-block and out-projection matmuls
for the previous q-block -- so the PE never idles waiting for exp. outproj(qb)
is delayed one block (attnT bufs=3) so the last, longest attention block still
has filler inventory.

Per-head softmax denominator comes from an extra ones column appended to V
(row 64 of the PV accumulator); normalization is reciprocal + broadcast mult.
Host sums the two head-group partials per batch and adds bo + bv @ wo
(the V bias folds out of the device since softmax rows sum to 1).
"""
from collections import deque

import numpy as np

import concourse.bass as bass
import concourse.bacc as bacc
import concourse.tile as tile
import concourse.mybir as mybir
from concourse.bass_utils import run_bass_kernel_spmd

B, T, C = 4, 2048, 1024
H, HS = 16, 64
NCORES = 8
HPC = 8            # heads per core
M = HPC * HS       # 512: per-core head dims
SCALE = HS ** -0.5

f32 = mybir.dt.float32
bf16 = mybir.dt.bfloat16
ADT = bf16           # device compute dtype (x, weights, attention internals)

TQ = 512           # tq block width
TK = 128           # tk block width
NQB = T // TQ      # 4
NKB = T // TK      # 16

_CACHED_NC = None


class _Body:
    def __init__(self, nc, tc, pools, aps):
        self.nc = nc
        self.tc = tc
        (self.pw, self.pq, self.px, self.ppt, self.pr, self.po, self.psp) = pools
        (self.xT_d, self.wq_d, self.wk_d, self.wv_d, self.wo_d,
         self.bq_d, self.bk_d, self.out_d) = aps
        self.Exp = mybir.ActivationFunctionType.Exp
        self.mult = mybir.AluOpType.mult
        self.add = mybir.AluOpType.add
        self.QT = {}
        self.attnT = {}
        self.fillers = deque()
        self.kv_fillers = deque()  # deferred KT/V proj units with a hard deadline
        self.tail_fillers = deque()  # units held for the final normalize chain

    # ---------- filler machinery ----------

    def pop_filler(self, n=1):
        for _ in range(n):
            if self.kv_fillers:
                self.kv_fillers.popleft()()
            elif self.fillers:
                self.fillers.popleft()()
            else:
                return

    def flush_kv(self):
        while self.kv_fillers:
            self.kv_fillers.popleft()()

    def flush_fillers(self):
        self.flush_kv()
        while self.fillers:
            self.fillers.popleft()()

    # ---------- prologue ----------

    def prologue(self):
        nc, pw = self.nc, self.pw
        self.KT = pw.tile([128, 4, T], ADT, tag="KT")
        self.V = pw.tile([128, NKB, HPC, HS + 1], ADT, tag="V")
        self.wq_sb = pw.tile([128, 8, M], ADT, tag="wq")
        self.wk_sb = pw.tile([128, 8, M], ADT, tag="wk")
        self.wv_sb = pw.tile([128, 8, M], ADT, tag="wv")
        self.wo_sb = pw.tile([128, 4, C], ADT, tag="wo")
        self.bq_sb = pw.tile([128, 4], f32, tag="bq")
        self.bk_sb = pw.tile([128, 4], f32, tag="bk")

        nc.gpsimd.memset(self.V[:, :, :, HS], 1.0)  # ones col
        self.zero_reg = nc.gpsimd.to_reg(0.0)  # cached affine_select fill
        self.xT_r = self.xT_d.rearrange("(co p) t -> p co t", p=128)
        self.wq_r = self.wq_d.rearrange("(co p) m -> p co m", p=128)
        self.wk_r = self.wk_d.rearrange("(co p) m -> p co m", p=128)
        self.wv_r = self.wv_d.rearrange("(co p) m -> p co m", p=128)

    # ---------- projections ----------

    def proj0(self):
        """tb=0 projection, eager, with per-chunk weight+x DMA interleave so
        the PE starts as soon as the first wq/x chunks land."""
        nc = self.nc
        QT = self.pq.tile([128, 4, TQ], ADT, tag="QT", bufs=2, name="QT_0")
        self.QT[0] = QT
        xin = self.px.tile([128, 8, TQ], ADT, tag="xin", name="xin_0")
        for cp in range(4):
            cs = bass.ds(2 * cp, 2)
            nc.sync.dma_start(self.wq_sb[:, cs, :], self.wq_r[:, cs, :])
            nc.sync.dma_start(xin[:, cs, :], self.xT_r[:, cs, bass.ds(0, TQ)])
            if cp == 0:
                # biases are only needed at the first epilogue; dispatch them
                # after the first weight/x chunks so the PE starts sooner
                nc.sync.dma_start(
                    self.bq_sb[:], self.bq_d.rearrange("(mo p) -> p mo", p=128))
                nc.sync.dma_start(
                    self.bk_sb[:], self.bk_d.rearrange("(mo p) -> p mo", p=128))
        for cp in range(2):
            cs = bass.ds(4 * cp, 4)
            nc.sync.dma_start(self.wk_sb[:, cs, :], self.wk_r[:, cs, :])
            nc.sync.dma_start(self.wv_sb[:, cs, :], self.wv_r[:, cs, :])
        for u in self._qt_units(0, QT, xin) + self._kv_units(0, xin):
            u()

    def stock_proj(self, tb):
        """Allocate xin, start its DMA, and queue proj matmuls as fillers.
        For the last t-block, KT/V units are deferred into attention(tb)
        itself (they are only consumed at its diagonal kb blocks)."""
        nc = self.nc
        QT = self.pq.tile([128, 4, TQ], ADT, tag="QT", bufs=2, name=f"QT_{tb}")
        self.QT[tb] = QT
        xin = self.px.tile([128, 8, TQ], ADT, tag="xin", name=f"xin_{tb}")
        nc.sync.dma_start(xin[:], self.xT_r[:, :, bass.ds(tb * TQ, TQ)])
        self.fillers.extend(self._qt_units(tb, QT, xin))
        if tb == NQB - 1:
            self.deferred_kv = self._kv_units(tb, xin)
        else:
            self.fillers.extend(self._kv_units(tb, xin))

    def _qt_units(self, tb, QT, xin):
        return self._wgroups(tb, xin, ((self.wq_sb, self.bq_sb, QT, 0),))

    def _kv_units(self, tb, xin):
        """KT groups + V groups, ordered so attention(tb)'s hp0 deadline
        (KT pair 0, then V kb blocks) is met first."""
        nc = self.nc
        units = self._wgroups(
            tb, xin, ((self.wk_sb, self.bk_sb, self.KT, tb * TQ),), mbs=(0,))
        for tv in range(4):
            cell = {}

            def mmv(ci, cell=cell, tv=tv, tb=tb, xin=xin):
                if ci == 0:
                    cell["ps"] = self.psp.tile(
                        [128, M], f32, tag="ps", name=f"pv_{tb}_{tv}")
                nc.tensor.matmul(
                    cell["ps"][:], xin[:, ci, bass.ts(tv, 128)],
                    self.wv_sb[:, ci, :], start=ci == 0, stop=ci == 7)

            def epv(cell=cell, tv=tv, tb=tb):
                kb = tb * 4 + tv
                nc.vector.tensor_copy(
                    self.V[:, kb, :, 0:HS],
                    cell["ps"][:].rearrange("p (h s) -> p h s", h=HPC))

            units.extend(lambda ci=ci, mmv=mmv: mmv(ci) for ci in range(8))
            units.append(epv)
        units.extend(self._wgroups(
            tb, xin, ((self.wk_sb, self.bk_sb, self.KT, tb * TQ),), mbs=(1, 2, 3)))
        return units

    def _wgroups(self, tb, xin, specs, mbs=(0, 1, 2, 3)):
        """Weight-projection matmul groups as single-matmul units + bias-add
        epilogue."""
        nc = self.nc
        units = []
        for w_sb, b_sb, dst, dsl in specs:
            for mb in mbs:
                cell = {}

                def mm(ci, cell=cell, w_sb=w_sb, mb=mb, tb=tb, xin=xin):
                    if ci == 0:
                        cell["ps"] = self.psp.tile(
                            [128, TQ], f32, tag="ps",
                            name=f"p_{tb}_{id(cell) % 97}_{mb}")
                    nc.tensor.matmul(
                        cell["ps"][:], w_sb[:, ci, bass.ts(mb, 128)],
                        xin[:, ci, :], start=ci == 0, stop=ci == 7)

                def ep(cell=cell, b_sb=b_sb, dst=dst, dsl=dsl, mb=mb):
                    nc.vector.tensor_tensor(
                        dst[:, mb, bass.ds(dsl, TQ)], cell["ps"][:],
                        b_sb[:, mb:mb + 1].to_broadcast((128, TQ)), self.add)

                units.extend(lambda ci=ci, mm=mm: mm(ci) for ci in range(8))
                units.append(ep)
        return units

    # ---------- out-projection ----------

    def _outproj_units(self, qb):
        """4 t-blocks x 2 column halves; one gathered DMA per t-block."""
        nc = self.nc
        attnT = self.attnT.pop(qb)
        units = []
        for tb2 in range(4):
            tt = qb * 4 + tb2
            cell = {}

            def mm2(cb, mo0, cell=cell, attnT=attnT, tb2=tb2, tt=tt):
                if mo0 == 0:
                    cell[cb] = self.psp.tile(
                        [128, 512], f32, tag="ps", name=f"po_{tt}_{cb}")
                for mo in (mo0, mo0 + 1):
                    nc.tensor.matmul(
                        cell[cb][:], attnT[:, mo, bass.ts(tb2, 128)],
                        self.wo_sb[:, mo, bass.ts(cb, 512)],
                        start=mo == 0, stop=mo == 3)

            def cp(cb, cell=cell, tt=tt, qb=qb):
                if cb == 0:
                    cell["o"] = self.po.tile([128, 2, 512], f32, tag="o",
                                             name=f"o_{tt}")
                nc.vector.tensor_copy(cell["o"][:, cb, :], cell[cb][:])
                if tt == T // 128 - 1:
                    # last t-block: DMA each half separately so the final
                    # transfer (and the end-of-kernel drain) starts sooner
                    nc.sync.dma_start(
                        self.out_d[bass.ts(tt, 128), bass.ts(cb, 512)],
                        cell["o"][:, cb, :])
                elif cb == 1:
                    nc.sync.dma_start(
                        self.out_d[bass.ts(tt, 128), :],
                        cell["o"].rearrange("p c n -> p (c n)"))

            for cb in range(2):
                units.append(lambda cb=cb, mm2=mm2: mm2(cb, 0))
                units.append(lambda cb=cb, mm2=mm2: mm2(cb, 2))
                units.append(lambda cb=cb, cp=cp: cp(cb))
        return units

    # ---------- attention ----------

    def attention(self, qb):
        nc = self.nc
        if qb == 0:
            nc.sync.dma_start(self.wo_sb[:], self.wo_d.rearrange(
                "(mo p) n -> p mo n", p=128))
        if qb + 1 < NQB:
            self.stock_proj(qb + 1)
        if qb == NQB - 1:
            self.kv_fillers.extend(self.deferred_kv)
            self.deferred_kv = []
        # outproj(0) fills qb1's stalls; outproj(1) and (2) fill qb3's (the
        # longest attention block, which has no next-proj inventory)
        if qb == 1:
            self.fillers.extend(self._outproj_units(0))
        elif qb == NQB - 1:
            for q in range(1, NQB - 1):
                self.fillers.extend(self._outproj_units(q))
        nkb = 4 * (qb + 1)
        events = max(1, 4 * (nkb - 1))
        rate = -(-len(self.fillers) // events)  # ceil
        QT = self.QT.pop(qb)
        attnT = self.pq.tile([128, 4, TQ], ADT, tag="attnT", bufs=3,
                             name=f"attnT_{qb}")
        self.attnT[qb] = attnT
        for hp in range(4):
            heads = (2 * hp, 2 * hp + 1)
            at_ps = {h: self.psp.tile([128, TQ], f32, tag="attn",
                                      name=f"attn_{qb}_{h}")
                     for h in heads}
            pts = {}

            def emit_pv(kb):
                s = kb - 4 * qb
                off = max(0, s) * 128
                w = TQ - off
                pt = pts.pop(kb)
                for i, h in enumerate(heads):
                    nc.tensor.matmul(
                        at_ps[h][0:HS + 1, bass.ds(off, w)],
                        self.V[:, kb, h, :], pt[:, i, 0:w],
                        start=kb == 0, stop=kb == nkb - 1)

            for kb in range(nkb):
                s = kb - 4 * qb   # >=0 on the diagonal staircase
                if s >= 0:
                    # diagonal blocks read this qb's own KT/V: deferred proj
                    # units must be emitted before their consumers
                    self.flush_kv()
                off = max(0, s) * 128
                w = TQ - off
                sc = self.psp.tile([128, 2, TQ], f32, tag="sc",
                                   name=f"sc_{qb}_{hp}_{kb}")
                for i, h in enumerate(heads):
                    hb = (h % 2) * 64
                    nc.tensor.matmul(
                        sc[:, i, 0:w],
                        self.KT[hb:hb + 64, h // 2, bass.ts(kb, TK)],
                        QT[hb:hb + 64, h // 2, bass.ds(off, w)],
                        start=True, stop=True)
                pt = self.ppt.tile([128, 2, TQ], ADT, tag="pt",
                                   name=f"pt_{qb}_{hp}_{kb}")
                pts[kb] = pt
                nc.scalar.activation(pt[:, :, 0:w], sc[:, :, 0:w], self.Exp,
                                     scale=SCALE)
                if s >= 0:
                    for i in range(2):
                        # keep upper triangle (incl diag), zero below
                        nc.gpsimd.affine_select(
                            out=pt[:, i, 0:128], in_=pt[:, i, 0:128],
                            compare_op=mybir.AluOpType.is_ge,
                            fill=self.zero_reg, base=0,
                            pattern=[[1, 128]], channel_multiplier=-1)
                if kb >= 1:
                    emit_pv(kb - 1)   # PV trails one block: scores/exp lead
                if self.kv_fillers:
                    self.pop_filler(7)
                else:
                    self.pop_filler(rate)
            emit_pv(nkb - 1)
            tail = qb == NQB - 1 and hp == 3
            self.pop_filler(2)
            dens = {}
            for h in heads:
                den64 = dens[h] = self.pr.tile([64, TQ], f32, tag="den64",
                                               name=f"d64_{qb}_{h}")
                nc.vector.tensor_copy(den64[0:1, :], at_ps[h][HS:HS + 1, :])
                nc.vector.reciprocal_approx_fast(out=den64[0:1, :],
                                                 in_=den64[0:1, :])
                nc.gpsimd.partition_broadcast(den64[:], den64[0:1, :])
                self.pop_filler(1)
            self.pop_filler(2)
            # on the last head pair, normalize in 128-col chunks so the final
            # outproj can start before the full-width mult finishes
            for t2 in (range(4) if tail else (slice(None),)):
                cols = bass.ts(t2, 128) if tail else slice(None)
                for h in heads:
                    hb = (h % 2) * 64
                    nc.vector.tensor_tensor(
                        attnT[hb:hb + 64, h // 2, cols],
                        at_ps[h][0:HS, cols], dens[h][:, cols], self.mult)
                self.pop_filler(1)
        self.flush_fillers()

    def emit(self):
        self.prologue()
        self.proj0()
        for qb in range(NQB):
            self.attention(qb)
        for u in self._outproj_units(NQB - 1):
            u()


def _build_nc(repeat=1):
    nc = bacc.Bacc("TRN2", target_bir_lowering=False, debug=False,
                   num_devices=NCORES)

    aps = (
        nc.dram_tensor("xT", [C, T], ADT, kind="ExternalInput").ap(),
        nc.dram_tensor("wq", [C, M], ADT, kind="ExternalInput").ap(),
        nc.dram_tensor("wk", [C, M], ADT, kind="ExternalInput").ap(),
        nc.dram_tensor("wv", [C, M], ADT, kind="ExternalInput").ap(),
        nc.dram_tensor("wo", [M, C], ADT, kind="ExternalInput").ap(),
        nc.dram_tensor("bq", [M], f32, kind="ExternalInput").ap(),
        nc.dram_tensor("bk", [M], f32, kind="ExternalInput").ap(),
        nc.dram_tensor("out", [T, C], f32, kind="ExternalOutput").ap(),
    )

    with tile.TileContext(nc) as tc:
        with tc.tile_pool(name="pw", bufs=1) as pw, \
             tc.tile_pool(name="pq", bufs=2) as pq, \
             tc.tile_pool(name="px", bufs=2) as px, \
             tc.tile_pool(name="ppt", bufs=3) as ppt, \
             tc.tile_pool(name="pr", bufs=2) as pr, \
             tc.tile_pool(name="po", bufs=6) as po, \
             tc.tile_pool(name="psp", bufs=2, space="PSUM") as psp:
            pools = (pw, pq, px, ppt, pr, po, psp)
            if repeat == 1:
                _Body(nc, tc, pools, aps).emit()
            else:
                with tc.For_i(0, repeat, 1):
                    _Body(nc, tc, pools, aps).emit()

    nc.finalize()
    return nc


def _get_nc():
    global _CACHED_NC
    if _CACHED_NC is None:
        _CACHED_NC = _build_nc()
    return _CACHED_NC


def make_in_maps(x, wq, wk, wv, wo, bq, bk):
    bf16_np = mybir.dt.np(ADT)
    in_maps = []
    for c in range(NCORES):
        b, g = c // 2, c % 2
        sl = slice(M * g, M * (g + 1))
        in_maps.append({
            "xT": np.ascontiguousarray(x[b].T).astype(bf16_np),
            "wq": np.ascontiguousarray(wq[:, sl]).astype(bf16_np),
            "wk": np.ascontiguousarray(wk[:, sl]).astype(bf16_np),
            "wv": np.ascontiguousarray(wv[:, sl]).astype(bf16_np),
            "wo": np.ascontiguousarray(wo[sl, :]).astype(bf16_np),
            "bq": np.ascontiguousarray(bq[sl]),
            "bk": np.ascontiguousarray(bk[sl]),
        })
    return in_maps


def kernel(**inputs):
    x = np.asarray(inputs["x"], dtype=np.float32)
    args = [np.asarray(inputs[k], dtype=np.float32)
            for k in ["wq", "wk", "wv", "wo", "bq", "bk"]]
    bv = np.asarray(inputs["bv"], dtype=np.float32)
    wo = args[3]
    bo = np.asarray(inputs["bo"], dtype=np.float32)

    in_maps = make_in_maps(x, *args)
    res = run_bass_kernel_spmd(_get_nc(), in_maps, core_ids=list(range(NCORES)))
    parts = [r["out"] for r in res.results]
    out = np.stack([parts[2 * b] + parts[2 * b + 1] for b in range(B)])
    # P @ (V + bv) == P @ V + bv  (softmax rows sum to 1), so bv folds into
    # a constant output offset bv @ wo, applied here with bo.
    out += bo + bv @ wo
    return out.astype(np.float32)


if __name__ == "__main__":
    nc = _build_nc()
    print("built ok, instructions:", len(nc.inst_map))
